# revision 9
# baseline (speedup 1.0000x reference)
"""MLA (CustomLlamaMLAForInfer) Trainium2 Bass kernel v2.

Sharding: hybrid batch x heads across 8 NeuronCores. Core c owns batch
b = c//4 and kv-head pair g = c%4 (kv heads {2g,2g+1}, q heads
{8g..8g+7}); it processes its batch's full 2048-token sequence and
produces a partial [2048, 4096] o_proj output; the host sums the 4
partials per batch (host work is not on the measured critical path).

Host folds the low-rank up-projections into the shared down-projection
(W_kc = Wupk_g @ Wdk, W_v = Wupv_g @ Wdk), so the device runs one fused
bf16 projection of hidden with columns [q 1024 | kc 128 | kr 128 | v 256].
q/k contraction dims use a permuted order pi = [rope_lo, rope_hi,
nope_lo, nope_hi] per head so rope/nope rows are contiguous (no scatter).

Device phases (single SPMD program, per-core weights differ):
  1. q-pass: qT[d, tok] per head tile; rope + 1/sqrt(d) folded in tables
  2. kv-pass: kT (roped rope rows + folded nope rows), v_tok in [tok, hd]
     layout (hid-stationary matmuls)
  3. causal attention per (head, q-block): scores_T = kT.T @ qT blocks,
     exp (scalar engine, bf16 out), diag mask, AV accumulate in PSUM;
     denominator: vector-accumulated p + one ones-matmul per q-block
  4. partial o_proj: out[tok, hid] += attn_T.T @ WoT_shard

All big matmuls in bf16 (1 cyc/row, FWL weight loads); PSUM accumulation
is fp32. Everything SBUF-resident between phases (no DRAM scratch).
"""

import numpy as np

HIDDEN = 4096
N_HEADS = 32
KV_HEADS = 8
HEAD_DIM = 128
LOW_RANK = 64
TOP_K_ROPE = 32
ROPE_THETA = 10000.0
B, S = 2, 2048
NCORES = 8
GPC = 2                       # kv heads per core
QT = 8                        # q-head tiles per core
QR = QT * HEAD_DIM            # q rows per core = 1024
W1C = QR + 64 * GPC + 64 * GPC + HEAD_DIM * GPC   # 1536 fused proj cols
KCOFF = QR                    # 1024
KROFF = QR + 64 * GPC         # 1152
VOFF = KROFF + 64 * GPC       # 1280
TB = 512                      # proj token block
QB = 512                      # attention q block
NTB = S // TB                 # 4
NQB = S // QB                 # 4
NJ = QB // 128                # 4
NKT = S // 128                # 16
HT = HIDDEN // 128            # 32

# pi: within-head dim order [rope_lo(0:32), rope_hi(64:96), nope_lo(32:64), nope_hi(96:128)]
PERM = np.concatenate([np.arange(0, 32), np.arange(64, 96),
                       np.arange(32, 64), np.arange(96, 128)])


def _rope_tables(seq_len):
    inv = 1.0 / (ROPE_THETA ** (np.arange(0, HEAD_DIM, 2, dtype=np.float32) / HEAD_DIM))
    pos = np.arange(seq_len, dtype=np.float32)
    fr = np.outer(pos, inv)
    emb = np.concatenate([fr, fr], axis=-1)          # [S, 128]
    return (np.cos(emb).T.astype(np.float32),        # [128, S] rows = dims
            np.sin(emb).T.astype(np.float32))


def build_program(trace_sim=False):
    from concourse import bacc, tile, mybir
    import concourse.bass as bass

    f32 = mybir.dt.float32
    bf16 = mybir.dt.bfloat16
    F32R = mybir.dt.float32r
    MS = bass.MemorySpace
    EXP = mybir.ActivationFunctionType.Exp

    nc = bacc.Bacc("TRN2", target_bir_lowering=False, debug=False,
                   num_devices=NCORES)

    def din(name, shape, dt=bf16):
        return nc.dram_tensor(name, shape, dt, kind="ExternalInput").ap()

    hidT = din("hidT", [HIDDEN, S])
    w1 = din("w1", [HIDDEN, W1C])          # fused proj weights, pre-transposed
    wo_t = din("wo_t", [QR, HIDDEN])
    qcos = din("qcos", [128, S])
    qsin = din("qsin", [128, S])
    kcos = din("kcos", [64 * GPC, S])
    ksin = din("ksin", [64 * GPC, S])
    masks = din("masks", [128, 2, 256])
    onesd = din("ones", [128, 1], f32)
    identd = din("ident", [128, 128])
    outp = nc.dram_tensor("out_part", [S, HIDDEN], f32, kind="ExternalOutput").ap()

    with tile.TileContext(nc, trace_sim=trace_sim) as tc:
        with tc.tile_pool(name="persist", bufs=1) as pers:
            # lo/hi token halves so attention qb0/1 doesn't depend on the
            # projection epilogue (deps are tile-granular)
            H2 = S // 2
            qTh = [pers.tile([128, QT, H2], bf16, tag=f"qT{_i}", name=f"qT{_i}")
                   for _i in range(2)]
            kTh = [pers.tile([128, GPC, H2], bf16, tag=f"kT{_i}", name=f"kT{_i}")
                   for _i in range(2)]
            vTh = [pers.tile([128, NKT // 2, GPC * HEAD_DIM], bf16,
                             tag=f"vT{_i}", name=f"vT{_i}") for _i in range(2)]

            masks_sb = pers.tile([128, 2, 256], bf16, tag="masks")
            nc.sync.dma_start(masks_sb[:], masks)
            ones_sb = pers.tile([128, 1], F32R, tag="ones")
            nc.sync.dma_start(ones_sb[:], onesd.bitcast(F32R))
            ident_sb = pers.tile([128, 128], bf16, tag="ident")
            nc.sync.dma_start(ident_sb[:], identd)

            # ---------------- phase 1+2: fused projections ----------------
            with tc.tile_pool(name="w1p", bufs=1) as wp, \
                 tc.tile_pool(name="tabs", bufs=1) as tabs, \
                 tc.tile_pool(name="hidp", bufs=6) as hp, \
                 tc.tile_pool(name="stg", bufs=2) as stg, \
                 tc.tile_pool(name="stg1", bufs=1) as stg1:
                # w1 chunks are DMA'd just-in-time (interleaved with hid) so
                # the first matmul doesn't wait behind the whole 12 MB load.
                w1t = [None] * HT

                def get_w1(t):
                    if w1t[t] is None:
                        wt = wp.tile([128, W1C], bf16, tag=f"w1_{t}",
                                     name=f"w1_{t}")
                        nc.sync.dma_start(wt[:], w1[t * 128:(t + 1) * 128, :])
                        w1t[t] = wt
                    return w1t[t]

                qcos_sb = tabs.tile([128, S], bf16, tag="qc")
                qsin_sb = tabs.tile([128, S], bf16, tag="qs")
                kcos_sb = tabs.tile([64 * GPC, S], bf16, tag="kc")
                ksin_sb = tabs.tile([64 * GPC, S], bf16, tag="ks")

                def load_tables():
                    nc.sync.dma_start(qcos_sb[:], qcos)
                    nc.sync.dma_start(qsin_sb[:], qsin)
                    nc.sync.dma_start(kcos_sb[:], kcos)
                    nc.sync.dma_start(ksin_sb[:], ksin)

                # ---- fused projection: q + kc/kr + v in one hid stream ----
                # TB2=256 token blocks; two accumulation groups share each
                # PSUM bank (the first group's start=True clears the bank,
                # the second rides it with start=False; only the last writer
                # sets stop).
                TB2 = 256
                NTB2 = S // TB2
                with tc.tile_pool(name="psF", bufs=5, space=MS.PSUM) as psF, \
                     tc.tile_pool(name="psKK", bufs=2, space=MS.PSUM) as psKK, \
                     tc.tile_pool(name="psVV", bufs=1, space=MS.PSUM) as psVV:
                    for blk in range(NTB2):
                        c0, c1 = blk * TB2, (blk + 1) * TB2
                        qpair = [psF.tile([128, 512], f32, tag="qp",
                                          name=f"qp{_m}") for _m in range(4)]
                        kk = psKK.tile([128, 512], f32, tag="kk")
                        vv = psVV.tile([128, 512], f32, tag="vv")
                        for tq in range(HT // 4):
                            ht = hp.tile([128, 4, TB2], bf16, tag="hid")
                            nc.sync.dma_start(
                                ht[:], hidT[tq * 512:(tq + 1) * 512, c0:c1]
                                .rearrange("(t p) w -> p t w", p=128))
                            for j in range(4):
                                t = tq * 4 + j
                                wt = get_w1(t)
                                first, last = (t == 0), (t == HT - 1)
                                for m in range(QT):
                                    half = m % 2
                                    nc.tensor.matmul(
                                        qpair[m // 2][:, half * 256:half * 256 + 256],
                                        wt[:, m * 128:(m + 1) * 128],
                                        ht[:, j, :],
                                        start=(first and half == 0),
                                        stop=(last and half == 1))
                                nc.tensor.matmul(
                                    kk[:, 0:256], wt[:, KCOFF:KCOFF + 128],
                                    ht[:, j, :],
                                    start=first, stop=False)
                                nc.tensor.matmul(
                                    kk[:, 256:512], wt[:, KROFF:KROFF + 128],
                                    ht[:, j, :],
                                    start=False, stop=last)
                                for sblk in range(2):
                                    nc.tensor.matmul(
                                        vv[:, sblk * 256:sblk * 256 + 256],
                                        ht[:, j, sblk * 128:(sblk + 1) * 128],
                                        wt[:, VOFF:VOFF + GPC * HEAD_DIM],
                                        start=(first and sblk == 0),
                                        stop=(last and sblk == 1))
                        if blk == 0:
                            load_tables()
                        # stage this block's outputs; rope/evict once per
                        # 512-token blockpair to halve small-DMA count
                        half = blk % 2
                        if half == 0:
                            qst = stg.tile([128, QT, 512], bf16, tag="qst")
                            krst = stg1.tile([128, 512], bf16, tag="krst")
                            kcst = stg1.tile([128, 512], bf16, tag="kcst")
                        hc = half * 256
                        for m in range(QT):
                            nc.scalar.copy(qst[:, m, hc:hc + 256],
                                           qpair[m // 2][:, (m % 2) * 256:(m % 2) * 256 + 256])
                        nc.scalar.copy(krst[:, hc:hc + 256], kk[:, 256:512])
                        nc.scalar.copy(kcst[:, hc:hc + 256], kk[:, 0:256])
                        for sblk in range(2):
                            slot = blk * 2 + sblk
                            nc.vector.tensor_copy(
                                vTh[slot // 8][:, slot % 8, :],
                                vv[:, sblk * 256:sblk * 256 + 256])
                        if half == 1:
                            bp = blk // 2          # blockpair id 0..3
                            p0 = bp * 512 - (bp // 2) * H2
                            hx = bp // 2           # lo/hi half index
                            qrot = stg1.tile([128, QT, 512], bf16, tag="qrot")
                            nc.scalar.dma_start(qrot[0:32], qst[32:64])
                            nc.scalar.dma_start(qrot[32:64], qst[0:32])
                            nc.scalar.dma_start(qrot[64:96], qst[96:128])
                            nc.scalar.dma_start(qrot[96:128], qst[64:96])
                            tc0 = bp * 512
                            for m in range(QT):
                                qd = qTh[hx][:, m, p0:p0 + 512]
                                nc.vector.tensor_mul(qd, qst[:, m, :],
                                                     qcos_sb[:, tc0:tc0 + 512])
                                nc.vector.tensor_mul(qrot[:, m, :], qrot[:, m, :],
                                                     qsin_sb[:, tc0:tc0 + 512])
                                nc.vector.tensor_add(qd, qd, qrot[:, m, :])
                            krot = stg1.tile([128, 512], bf16, tag="krot")
                            nc.sync.dma_start(krot[0:32, :], krst[32:64, :])
                            nc.sync.dma_start(krot[32:64, :], krst[0:32, :])
                            nc.sync.dma_start(krot[64:96, :], krst[96:128, :])
                            nc.sync.dma_start(krot[96:128, :], krst[64:96, :])
                            kst = stg1.tile([128, 512], bf16, tag="kst")
                            nc.vector.tensor_mul(kst[:], krst[:],
                                                 kcos_sb[:, tc0:tc0 + 512])
                            nc.vector.tensor_mul(krot[:], krot[:],
                                                 ksin_sb[:, tc0:tc0 + 512])
                            nc.vector.tensor_add(kst[:], kst[:], krot[:])
                            nc.sync.dma_start(kTh[hx][0:64, 0, p0:p0 + 512], kst[0:64, :])
                            nc.sync.dma_start(kTh[hx][0:64, 1, p0:p0 + 512], kst[64:128, :])
                            nc.sync.dma_start(kTh[hx][64:128, 0, p0:p0 + 512], kcst[0:64, :])
                            nc.sync.dma_start(kTh[hx][64:128, 1, p0:p0 + 512], kcst[64:128, :])

            # ---------------- phases 3+4 ----------------
            with tc.tile_pool(name="pt", bufs=10) as ptp, \
                 tc.tile_pool(name="sm", bufs=2) as smp, \
                 tc.tile_pool(name="attnp", bufs=1) as ap_, \
                 tc.tile_pool(name="wop", bufs=1) as wop:
                attn_sb = ap_.tile([128, QT, S], bf16, tag="attn")
                wo_sb = []
                for h in range(QT):
                    wt = wop.tile([128, HIDDEN], bf16, tag=f"wo_{h}")
                    nc.scalar.dma_start(wt[:], wo_t[h * 128:(h + 1) * 128, :])
                    wo_sb.append(wt)

                # Attention zipped with o_proj: o_proj matmul jobs for
                # q-block qb-1 are interleaved into qb's kt loop to fill the
                # PE slots that would otherwise stall on the scalar exp.
                with tc.tile_pool(name="psS", bufs=3, space=MS.PSUM) as psS, \
                     tc.tile_pool(name="psO", bufs=2, space=MS.PSUM) as psO, \
                     tc.tile_pool(name="psU", bufs=1, space=MS.PSUM) as psU, \
                     tc.tile_pool(name="st4", bufs=4) as st4, \
                     tc.tile_pool(name="ps4", bufs=2, space=MS.PSUM) as ps4, \
                     tc.tile_pool(name="accp", bufs=2) as accp:
                    ojobs = []

                    def make_ojobs(qb):
                        for T in range(qb * 2, (qb + 1) * 2):
                            holder = {}
                            for half in range(2):
                                for n in range(HIDDEN // 512):
                                    def job(T=T, n=n, half=half, holder=holder):
                                        if half == 0:
                                            holder[n] = ps4.tile(
                                                [128, 512], f32, tag="ps",
                                                name=f"ps{T}_{n}")
                                        ps = holder[n]
                                        for h2 in range(4 * half, 4 * half + 4):
                                            nc.tensor.matmul(
                                                ps[:],
                                                attn_sb[:, h2, T * 128:(T + 1) * 128],
                                                wo_sb[h2][:, n * 512:(n + 1) * 512],
                                                start=(h2 == 0), stop=(h2 == QT - 1))
                                        if half == 1:
                                            osb = st4.tile([128, 512], f32, tag="osb")
                                            nc.vector.tensor_copy(osb[:], ps[:])
                                            nc.sync.dma_start(
                                                outp[T * 128:(T + 1) * 128,
                                                     n * 512:(n + 1) * 512],
                                                osb[:])
                                    ojobs.append(job)

                    def drain_ojob():
                        if ojobs:
                            ojobs.pop(0)()

                    QB2 = 256
                    for qb in range(NQB * 2):
                        tq0 = qb * QB2
                        qh_, qc0 = qb // 4, (qb % 4) * QB2
                        for h in range(QT):
                            gl = h // 4
                            ops = psO.tile([128, QB2], f32, tag="ops")
                            acc = accp.tile([128, QB2], F32R, tag="acc")
                            npair = qb + 1
                            scps = {}

                            def emit_pair(pi, h=h, gl=gl, qb=qb, scps=scps,
                                          qh_=qh_, qc0=qc0):
                                scp = psS.tile([128, 512], f32, tag="scp",
                                               name=f"scp{h}_{qb}_{pi}")
                                diag = (pi == qb)
                                for u in range(2):
                                    kt = 2 * pi + u
                                    grp_last = (u == 1 and not diag)
                                    nc.tensor.matmul(
                                        scp[:, u * 256:u * 256 + 256],
                                        kTh[kt // 8][:, gl, (kt % 8) * 128:(kt % 8) * 128 + 128],
                                        qTh[qh_][:, h, qc0:qc0 + QB2],
                                        start=(u == 0), stop=grp_last)
                                if diag:
                                    for u in range(2):
                                        nc.tensor.matmul(
                                            scp[:, u * 256:u * 256 + 256],
                                            ident_sb[:], masks_sb[:, u, :],
                                            start=False, stop=(u == 1))
                                scps[pi] = scp

                            emit_pair(0)
                            if npair > 1:
                                emit_pair(1)
                            for pi in range(npair):
                                if pi + 2 < npair:
                                    emit_pair(pi + 2)
                                scp = scps.pop(pi)
                                ptile = ptp.tile([128, 512], bf16, tag="pt")
                                nc.scalar.activation(ptile[:], scp[:], EXP)
                                for u in range(2):
                                    kt = 2 * pi + u
                                    nc.tensor.matmul(
                                        ops[:],
                                        vTh[kt // 8][:, kt % 8,
                                                     gl * HEAD_DIM:(gl + 1) * HEAD_DIM],
                                        ptile[:, u * 256:u * 256 + 256],
                                        start=(kt == 0), stop=(kt == 2 * npair - 1))
                                if pi == 0:
                                    nc.vector.tensor_add(acc[:], ptile[:, 0:256],
                                                         ptile[:, 256:512])
                                else:
                                    ptmp = smp.tile([128, QB2], F32R, tag="ptmp")
                                    nc.vector.tensor_add(ptmp[:], ptile[:, 0:256],
                                                         ptile[:, 256:512])
                                    nc.vector.tensor_add(acc[:], acc[:], ptmp[:])
                                drain_ojob()
                            sps = psU.tile([1, QB2], f32, tag="sps")
                            nc.tensor.matmul(sps[:], ones_sb[:], acc[:],
                                             start=True, stop=True)
                            rec = smp.tile([1, QB2], f32, tag="rec")
                            nc.vector.reciprocal_approx_fast(out=rec[:], in_=sps[:])
                            rb = smp.tile([128, QB2], f32, tag="rb")
                            nc.gpsimd.partition_broadcast(rb[:], rec[:])
                            nc.vector.tensor_mul(
                                attn_sb[:, h, tq0:tq0 + QB2],
                                ops[:], rb[:])
                        make_ojobs(qb)
                    while ojobs:
                        drain_ojob()

    nc.compile()
    return nc


def make_in_maps(hidden_states, Wq, Wkr, Wdk, Wupk, Wupv, Wo):
    """Host-side sharding + layout prep (off the measured critical path)."""
    import ml_dtypes
    bf = ml_dtypes.bfloat16
    scale = np.float32(1.0 / np.sqrt(np.float32(HEAD_DIM)))

    hidden_states = np.asarray(hidden_states, np.float32)
    Wq = np.asarray(Wq, np.float32)
    Wkr = np.asarray(Wkr, np.float32)
    Wdk = np.asarray(Wdk, np.float32)
    Wupk = np.asarray(Wupk, np.float32)
    Wupv = np.asarray(Wupv, np.float32)
    Wo = np.asarray(Wo, np.float32)

    cos_t, sin_t = _rope_tables(S)                     # [128, S], rows = dims
    sgn = np.concatenate([-np.ones(32), np.ones(32),
                          -np.ones(32), np.ones(32)]).astype(np.float32)
    qcos = (cos_t[PERM] * scale).astype(bf)
    qsin = (sin_t[PERM] * sgn[:, None] * scale).astype(bf)
    rope_rows = np.concatenate([np.arange(0, 32), np.arange(64, 96)])
    ksgn = np.concatenate([-np.ones(32), np.ones(32)]).astype(np.float32)
    kcos1 = cos_t[rope_rows]                           # [64, S]
    ksin1 = sin_t[rope_rows] * ksgn[:, None]
    kcos = np.tile(kcos1, (GPC, 1)).astype(bf)
    ksin = np.tile(ksin1, (GPC, 1)).astype(bf)

    k_idx = np.arange(128)[:, None]
    q_idx = np.arange(QB)[None, :]
    # -30 bias on future (disallowed) slots, 0 on allowed: added to scores
    q_idx = np.arange(256)[None, :]
    masks = np.stack(
        [np.where(q_idx >= j * 128 + k_idx, 0.0, -30.0).astype(np.float32)
         for j in range(2)],
        axis=1).astype(bf)                             # [128, 2, 256]

    hidT = [np.ascontiguousarray(
        hidden_states[b].reshape(S, HIDDEN).T).astype(bf) for b in range(B)]

    in_maps = []
    for c in range(NCORES):
        b, g = divmod(c, 4)
        # q rows: heads 8g..8g+7, pi-permuted within each head
        wq_rows = np.concatenate(
            [Wq[(8 * g + h) * 128:(8 * g + h) * 128 + 128][PERM]
             for h in range(QT)], axis=0)              # [1024, 4096]
        # folded nope-key rows (pi nope order == Wupk row order per head)
        wkc = Wupk[128 * g:128 * g + 128] @ Wdk        # [128, 4096]
        # rope-key rows (pi rope order == Wkr row order per head)
        wkr = Wkr[128 * g:128 * g + 128]               # [128, 4096]
        # folded v rows, canonical head-dim order
        wv = Wupv[256 * g:256 * g + 256] @ Wdk         # [256, 4096]
        w1 = np.ascontiguousarray(
            np.concatenate([wq_rows, wkc, wkr, wv], axis=0).T).astype(bf)
        wo_c = np.ascontiguousarray(
            Wo[:, QR * g:QR * (g + 1)].T).astype(bf)   # [1024, 4096]
        in_maps.append({
            "hidT": hidT[b], "w1": w1, "wo_t": wo_c,
            "qcos": qcos, "qsin": qsin, "kcos": kcos, "ksin": ksin,
            "masks": masks, "ones": np.ones((128, 1), np.float32),
            "ident": np.eye(128, dtype=np.float32).astype(bf),
        })
    return in_maps


def combine_outputs(results):
    outs = []
    for b in range(B):
        o = results[4 * b]["out_part"].astype(np.float32)
        for g in range(1, 4):
            o = o + results[4 * b + g]["out_part"]
        outs.append(o)
    return np.stack(outs, axis=0).reshape(B, S, HIDDEN).astype(np.float32)


_NC_CACHE = {}


def _get_program(key=0):
    if key not in _NC_CACHE:
        _NC_CACHE[key] = build_program()
    return _NC_CACHE[key]


def kernel(hidden_states, Wq, Wkr, Wdk, Wupk, Wupv, Wo):
    from concourse.bass_utils import run_bass_kernel_spmd

    in_maps = make_in_maps(hidden_states, Wq, Wkr, Wdk, Wupk, Wupv, Wo)
    nc = _get_program()
    res = run_bass_kernel_spmd(nc, in_maps, list(range(NCORES)))
    return combine_outputs(res.results)


# revision 10
# speedup vs baseline: 1.0114x; 1.0114x over previous
"""MLA (CustomLlamaMLAForInfer) Trainium2 Bass kernel v2.

Sharding: hybrid batch x heads across 8 NeuronCores. Core c owns batch
b = c//4 and kv-head pair g = c%4 (kv heads {2g,2g+1}, q heads
{8g..8g+7}); it processes its batch's full 2048-token sequence and
produces a partial [2048, 4096] o_proj output; the host sums the 4
partials per batch (host work is not on the measured critical path).

Host folds the low-rank up-projections into the shared down-projection
(W_kc = Wupk_g @ Wdk, W_v = Wupv_g @ Wdk), so the device runs one fused
bf16 projection of hidden with columns [q 1024 | kc 128 | kr 128 | v 256].
q/k contraction dims use a permuted order pi = [rope_lo, rope_hi,
nope_lo, nope_hi] per head so rope/nope rows are contiguous (no scatter).

Device phases (single SPMD program, per-core weights differ):
  1. fused projection, one pass over hidden (256-token blocks, 12 bf16
     matmuls per hidden tile; two accumulation groups share each PSUM
     bank): q (rope applied at evict), kT rows, v in [tok, hd] layout.
     Evictions are batched per 512-token blockpair; qT/kT/vT are split
     into lo/hi token halves so attention never waits on the epilogue.
  2. causal attention (q-blocks of 256, k-tiles paired two-per-PSUM-bank
     so one scalar-engine exp covers both; causal mask applied as a -30
     bias matmul into the scores PSUM; softmax denominator via DVE
     accumulation + one ones-matmul; reciprocal_approx_fast), zipped
     with the partial o_proj: o_proj matmul jobs of the previous q-block
     fill the PE slots that would otherwise stall on exp.

All matmuls in bf16 (1 cyc/row, FWL weight loads); PSUM accumulation is
fp32. Everything SBUF-resident between phases (no DRAM scratch).
"""

import numpy as np

HIDDEN = 4096
N_HEADS = 32
KV_HEADS = 8
HEAD_DIM = 128
LOW_RANK = 64
TOP_K_ROPE = 32
ROPE_THETA = 10000.0
B, S = 2, 2048
NCORES = 8
GPC = 2                       # kv heads per core
QT = 8                        # q-head tiles per core
QR = QT * HEAD_DIM            # q rows per core = 1024
W1C = QR + 64 * GPC + 64 * GPC + HEAD_DIM * GPC   # 1536 fused proj cols
KCOFF = QR                    # 1024
KROFF = QR + 64 * GPC         # 1152
VOFF = KROFF + 64 * GPC       # 1280
TB = 512                      # proj token block
QB = 512                      # attention q block
NTB = S // TB                 # 4
NQB = S // QB                 # 4
NJ = QB // 128                # 4
NKT = S // 128                # 16
HT = HIDDEN // 128            # 32

# pi: within-head dim order [rope_lo(0:32), rope_hi(64:96), nope_lo(32:64), nope_hi(96:128)]
PERM = np.concatenate([np.arange(0, 32), np.arange(64, 96),
                       np.arange(32, 64), np.arange(96, 128)])


def _rope_tables(seq_len):
    inv = 1.0 / (ROPE_THETA ** (np.arange(0, HEAD_DIM, 2, dtype=np.float32) / HEAD_DIM))
    pos = np.arange(seq_len, dtype=np.float32)
    fr = np.outer(pos, inv)
    emb = np.concatenate([fr, fr], axis=-1)          # [S, 128]
    return (np.cos(emb).T.astype(np.float32),        # [128, S] rows = dims
            np.sin(emb).T.astype(np.float32))


def build_program(trace_sim=False):
    from concourse import bacc, tile, mybir
    import concourse.bass as bass

    f32 = mybir.dt.float32
    bf16 = mybir.dt.bfloat16
    F32R = mybir.dt.float32r
    MS = bass.MemorySpace
    EXP = mybir.ActivationFunctionType.Exp

    nc = bacc.Bacc("TRN2", target_bir_lowering=False, debug=False,
                   num_devices=NCORES)

    def din(name, shape, dt=bf16):
        return nc.dram_tensor(name, shape, dt, kind="ExternalInput").ap()

    hidT = din("hidT", [HIDDEN, S])
    w1 = din("w1", [HIDDEN, W1C])          # fused proj weights, pre-transposed
    wo_t = din("wo_t", [QR, HIDDEN])
    qcos = din("qcos", [128, S])
    qsin = din("qsin", [128, S])
    kcos = din("kcos", [64 * GPC, S])
    ksin = din("ksin", [64 * GPC, S])
    masks = din("masks", [128, 2, 256])
    onesd = din("ones", [128, 1], f32)
    identd = din("ident", [128, 128])
    outp = nc.dram_tensor("out_part", [S, HIDDEN], f32, kind="ExternalOutput").ap()

    with tile.TileContext(nc, trace_sim=trace_sim) as tc:
        with tc.tile_pool(name="persist", bufs=1) as pers:
            # lo/hi token halves so attention qb0/1 doesn't depend on the
            # projection epilogue (deps are tile-granular)
            H2 = S // 2
            qTh = [pers.tile([128, QT, H2], bf16, tag=f"qT{_i}", name=f"qT{_i}")
                   for _i in range(2)]
            kTh = [pers.tile([128, GPC, H2], bf16, tag=f"kT{_i}", name=f"kT{_i}")
                   for _i in range(2)]
            vTh = [pers.tile([128, NKT // 2, GPC * HEAD_DIM], bf16,
                             tag=f"vT{_i}", name=f"vT{_i}") for _i in range(2)]

            masks_sb = pers.tile([128, 2, 256], bf16, tag="masks")
            nc.sync.dma_start(masks_sb[:], masks)
            ones_sb = pers.tile([128, 1], F32R, tag="ones")
            nc.sync.dma_start(ones_sb[:], onesd.bitcast(F32R))
            ident_sb = pers.tile([128, 128], bf16, tag="ident")
            nc.sync.dma_start(ident_sb[:], identd)

            # ---------------- phase 1+2: fused projections ----------------
            with tc.tile_pool(name="w1p", bufs=1) as wp, \
                 tc.tile_pool(name="tabs", bufs=1) as tabs, \
                 tc.tile_pool(name="hidp", bufs=6) as hp, \
                 tc.tile_pool(name="stg", bufs=2) as stg, \
                 tc.tile_pool(name="stg1", bufs=1) as stg1:
                # w1 chunks are DMA'd just-in-time (interleaved with hid) so
                # the first matmul doesn't wait behind the whole 12 MB load.
                w1t = [None] * HT

                def get_w1(t):
                    if w1t[t] is None:
                        wt = wp.tile([128, W1C], bf16, tag=f"w1_{t}",
                                     name=f"w1_{t}")
                        nc.sync.dma_start(wt[:], w1[t * 128:(t + 1) * 128, :])
                        w1t[t] = wt
                    return w1t[t]

                qcos_sb = tabs.tile([128, S], bf16, tag="qc")
                qsin_sb = tabs.tile([128, S], bf16, tag="qs")
                kcos_sb = tabs.tile([64 * GPC, S], bf16, tag="kc")
                ksin_sb = tabs.tile([64 * GPC, S], bf16, tag="ks")

                def load_tables():
                    nc.sync.dma_start(qcos_sb[:], qcos)
                    nc.sync.dma_start(qsin_sb[:], qsin)
                    nc.sync.dma_start(kcos_sb[:], kcos)
                    nc.sync.dma_start(ksin_sb[:], ksin)

                # ---- fused projection: q + kc/kr + v in one hid stream ----
                # TB2=256 token blocks; two accumulation groups share each
                # PSUM bank (the first group's start=True clears the bank,
                # the second rides it with start=False; only the last writer
                # sets stop).
                TB2 = 256
                NTB2 = S // TB2
                with tc.tile_pool(name="psF", bufs=5, space=MS.PSUM) as psF, \
                     tc.tile_pool(name="psKK", bufs=2, space=MS.PSUM) as psKK, \
                     tc.tile_pool(name="psVV", bufs=1, space=MS.PSUM) as psVV:
                    for blk in range(NTB2):
                        c0, c1 = blk * TB2, (blk + 1) * TB2
                        qpair = [psF.tile([128, 512], f32, tag="qp",
                                          name=f"qp{_m}") for _m in range(4)]
                        kk = psKK.tile([128, 512], f32, tag="kk")
                        vv = psVV.tile([128, 512], f32, tag="vv")
                        for tq in range(HT // 4):
                            ht = hp.tile([128, 4, TB2], bf16, tag="hid")
                            nc.sync.dma_start(
                                ht[:], hidT[tq * 512:(tq + 1) * 512, c0:c1]
                                .rearrange("(t p) w -> p t w", p=128))
                            for j in range(4):
                                t = tq * 4 + j
                                wt = get_w1(t)
                                first, last = (t == 0), (t == HT - 1)
                                for m in range(QT):
                                    half = m % 2
                                    nc.tensor.matmul(
                                        qpair[m // 2][:, half * 256:half * 256 + 256],
                                        wt[:, m * 128:(m + 1) * 128],
                                        ht[:, j, :],
                                        start=(first and half == 0),
                                        stop=(last and half == 1))
                                nc.tensor.matmul(
                                    kk[:, 0:256], wt[:, KCOFF:KCOFF + 128],
                                    ht[:, j, :],
                                    start=first, stop=False)
                                nc.tensor.matmul(
                                    kk[:, 256:512], wt[:, KROFF:KROFF + 128],
                                    ht[:, j, :],
                                    start=False, stop=last)
                                for sblk in range(2):
                                    nc.tensor.matmul(
                                        vv[:, sblk * 256:sblk * 256 + 256],
                                        ht[:, j, sblk * 128:(sblk + 1) * 128],
                                        wt[:, VOFF:VOFF + GPC * HEAD_DIM],
                                        start=(first and sblk == 0),
                                        stop=(last and sblk == 1))
                        if blk == 0:
                            load_tables()
                        # stage this block's outputs; rope/evict once per
                        # 512-token blockpair to halve small-DMA count
                        half = blk % 2
                        if half == 0:
                            qst = stg.tile([128, QT, 512], bf16, tag="qst")
                            krst = stg1.tile([128, 512], bf16, tag="krst")
                            kcst = stg1.tile([128, 512], bf16, tag="kcst")
                        hc = half * 256
                        for m in range(QT):
                            nc.scalar.copy(qst[:, m, hc:hc + 256],
                                           qpair[m // 2][:, (m % 2) * 256:(m % 2) * 256 + 256])
                        nc.scalar.copy(krst[:, hc:hc + 256], kk[:, 256:512])
                        nc.scalar.copy(kcst[:, hc:hc + 256], kk[:, 0:256])
                        for sblk in range(2):
                            slot = blk * 2 + sblk
                            nc.vector.tensor_copy(
                                vTh[slot // 8][:, slot % 8, :],
                                vv[:, sblk * 256:sblk * 256 + 256])
                        if half == 1:
                            bp = blk // 2          # blockpair id 0..3
                            p0 = bp * 512 - (bp // 2) * H2
                            hx = bp // 2           # lo/hi half index
                            qrot = stg1.tile([128, QT, 512], bf16, tag="qrot")
                            nc.scalar.dma_start(qrot[0:32], qst[32:64])
                            nc.scalar.dma_start(qrot[32:64], qst[0:32])
                            nc.scalar.dma_start(qrot[64:96], qst[96:128])
                            nc.scalar.dma_start(qrot[96:128], qst[64:96])
                            tc0 = bp * 512
                            for m in range(QT):
                                qd = qTh[hx][:, m, p0:p0 + 512]
                                nc.vector.tensor_mul(qd, qst[:, m, :],
                                                     qcos_sb[:, tc0:tc0 + 512])
                                nc.vector.tensor_mul(qrot[:, m, :], qrot[:, m, :],
                                                     qsin_sb[:, tc0:tc0 + 512])
                                nc.vector.tensor_add(qd, qd, qrot[:, m, :])
                            krot = stg1.tile([128, 512], bf16, tag="krot")
                            nc.sync.dma_start(krot[0:32, :], krst[32:64, :])
                            nc.sync.dma_start(krot[32:64, :], krst[0:32, :])
                            nc.sync.dma_start(krot[64:96, :], krst[96:128, :])
                            nc.sync.dma_start(krot[96:128, :], krst[64:96, :])
                            kst = stg1.tile([128, 512], bf16, tag="kst")
                            nc.vector.tensor_mul(kst[:], krst[:],
                                                 kcos_sb[:, tc0:tc0 + 512])
                            nc.vector.tensor_mul(krot[:], krot[:],
                                                 ksin_sb[:, tc0:tc0 + 512])
                            nc.vector.tensor_add(kst[:], kst[:], krot[:])
                            nc.sync.dma_start(kTh[hx][0:64, 0, p0:p0 + 512], kst[0:64, :])
                            nc.sync.dma_start(kTh[hx][0:64, 1, p0:p0 + 512], kst[64:128, :])
                            nc.sync.dma_start(kTh[hx][64:128, 0, p0:p0 + 512], kcst[0:64, :])
                            nc.sync.dma_start(kTh[hx][64:128, 1, p0:p0 + 512], kcst[64:128, :])

            # ---------------- phases 3+4 ----------------
            with tc.tile_pool(name="pt", bufs=10) as ptp, \
                 tc.tile_pool(name="sm", bufs=2) as smp, \
                 tc.tile_pool(name="attnp", bufs=1) as ap_, \
                 tc.tile_pool(name="wop", bufs=1) as wop:
                attn_sb = ap_.tile([128, QT, S], bf16, tag="attn")
                wo_sb = []
                for h in range(QT):
                    wt = wop.tile([128, HIDDEN], bf16, tag=f"wo_{h}")
                    nc.scalar.dma_start(wt[:], wo_t[h * 128:(h + 1) * 128, :])
                    wo_sb.append(wt)

                # Attention zipped with o_proj: o_proj matmul jobs for
                # q-block qb-1 are interleaved into qb's kt loop to fill the
                # PE slots that would otherwise stall on the scalar exp.
                with tc.tile_pool(name="psS", bufs=3, space=MS.PSUM) as psS, \
                     tc.tile_pool(name="psO", bufs=2, space=MS.PSUM) as psO, \
                     tc.tile_pool(name="psU", bufs=1, space=MS.PSUM) as psU, \
                     tc.tile_pool(name="st4", bufs=4) as st4, \
                     tc.tile_pool(name="ps4", bufs=2, space=MS.PSUM) as ps4, \
                     tc.tile_pool(name="accp", bufs=2) as accp:
                    ojobs = []

                    def make_ojobs(qb):
                        for T in range(qb * 2, (qb + 1) * 2):
                            holder = {}
                            for half in range(2):
                                for n in range(HIDDEN // 512):
                                    def job(T=T, n=n, half=half, holder=holder):
                                        if half == 0:
                                            holder[n] = ps4.tile(
                                                [128, 512], f32, tag="ps",
                                                name=f"ps{T}_{n}")
                                        ps = holder[n]
                                        for h2 in range(4 * half, 4 * half + 4):
                                            nc.tensor.matmul(
                                                ps[:],
                                                attn_sb[:, h2, T * 128:(T + 1) * 128],
                                                wo_sb[h2][:, n * 512:(n + 1) * 512],
                                                start=(h2 == 0), stop=(h2 == QT - 1))
                                        if half == 1:
                                            osb = st4.tile([128, 512], f32, tag="osb")
                                            nc.vector.tensor_copy(osb[:], ps[:])
                                            nc.sync.dma_start(
                                                outp[T * 128:(T + 1) * 128,
                                                     n * 512:(n + 1) * 512],
                                                osb[:])
                                    ojobs.append(job)

                    def drain_ojob():
                        if ojobs:
                            ojobs.pop(0)()

                    QB2 = 256
                    for qb in range(NQB * 2):
                        tq0 = qb * QB2
                        qh_, qc0 = qb // 4, (qb % 4) * QB2
                        for h in range(QT):
                            gl = h // 4
                            ops = psO.tile([128, QB2], f32, tag="ops")
                            acc = accp.tile([128, QB2], F32R, tag="acc")
                            npair = qb + 1
                            scps = {}

                            def emit_pair(pi, h=h, gl=gl, qb=qb, scps=scps,
                                          qh_=qh_, qc0=qc0):
                                scp = psS.tile([128, 512], f32, tag="scp",
                                               name=f"scp{h}_{qb}_{pi}")
                                diag = (pi == qb)
                                for u in range(2):
                                    kt = 2 * pi + u
                                    grp_last = (u == 1 and not diag)
                                    nc.tensor.matmul(
                                        scp[:, u * 256:u * 256 + 256],
                                        kTh[kt // 8][:, gl, (kt % 8) * 128:(kt % 8) * 128 + 128],
                                        qTh[qh_][:, h, qc0:qc0 + QB2],
                                        start=(u == 0), stop=grp_last)
                                if diag:
                                    for u in range(2):
                                        nc.tensor.matmul(
                                            scp[:, u * 256:u * 256 + 256],
                                            ident_sb[:], masks_sb[:, u, :],
                                            start=False, stop=(u == 1))
                                scps[pi] = scp

                            emit_pair(0)
                            if npair > 1:
                                emit_pair(1)
                            for pi in range(npair):
                                if pi + 2 < npair:
                                    emit_pair(pi + 2)
                                scp = scps.pop(pi)
                                ptile = ptp.tile([128, 512], bf16, tag="pt")
                                nc.scalar.activation(ptile[:], scp[:], EXP)
                                for u in range(2):
                                    kt = 2 * pi + u
                                    nc.tensor.matmul(
                                        ops[:],
                                        vTh[kt // 8][:, kt % 8,
                                                     gl * HEAD_DIM:(gl + 1) * HEAD_DIM],
                                        ptile[:, u * 256:u * 256 + 256],
                                        start=(kt == 0), stop=(kt == 2 * npair - 1))
                                if pi == 0:
                                    nc.vector.tensor_add(acc[:], ptile[:, 0:256],
                                                         ptile[:, 256:512])
                                else:
                                    ptmp = smp.tile([128, QB2], F32R, tag="ptmp")
                                    nc.vector.tensor_add(ptmp[:], ptile[:, 0:256],
                                                         ptile[:, 256:512])
                                    nc.vector.tensor_add(acc[:], acc[:], ptmp[:])
                                drain_ojob()
                            sps = psU.tile([1, QB2], f32, tag="sps")
                            nc.tensor.matmul(sps[:], ones_sb[:], acc[:],
                                             start=True, stop=True)
                            rec = smp.tile([1, QB2], f32, tag="rec")
                            nc.vector.reciprocal_approx_fast(out=rec[:], in_=sps[:])
                            rb = smp.tile([128, QB2], f32, tag="rb")
                            nc.gpsimd.partition_broadcast(rb[:], rec[:])
                            nc.vector.tensor_mul(
                                attn_sb[:, h, tq0:tq0 + QB2],
                                ops[:], rb[:])
                        make_ojobs(qb)
                    while ojobs:
                        drain_ojob()

    nc.compile()
    return nc


def make_in_maps(hidden_states, Wq, Wkr, Wdk, Wupk, Wupv, Wo):
    """Host-side sharding + layout prep (off the measured critical path)."""
    import ml_dtypes
    bf = ml_dtypes.bfloat16
    scale = np.float32(1.0 / np.sqrt(np.float32(HEAD_DIM)))

    hidden_states = np.asarray(hidden_states, np.float32)
    Wq = np.asarray(Wq, np.float32)
    Wkr = np.asarray(Wkr, np.float32)
    Wdk = np.asarray(Wdk, np.float32)
    Wupk = np.asarray(Wupk, np.float32)
    Wupv = np.asarray(Wupv, np.float32)
    Wo = np.asarray(Wo, np.float32)

    cos_t, sin_t = _rope_tables(S)                     # [128, S], rows = dims
    sgn = np.concatenate([-np.ones(32), np.ones(32),
                          -np.ones(32), np.ones(32)]).astype(np.float32)
    qcos = (cos_t[PERM] * scale).astype(bf)
    qsin = (sin_t[PERM] * sgn[:, None] * scale).astype(bf)
    rope_rows = np.concatenate([np.arange(0, 32), np.arange(64, 96)])
    ksgn = np.concatenate([-np.ones(32), np.ones(32)]).astype(np.float32)
    kcos1 = cos_t[rope_rows]                           # [64, S]
    ksin1 = sin_t[rope_rows] * ksgn[:, None]
    kcos = np.tile(kcos1, (GPC, 1)).astype(bf)
    ksin = np.tile(ksin1, (GPC, 1)).astype(bf)

    k_idx = np.arange(128)[:, None]
    q_idx = np.arange(QB)[None, :]
    # -30 bias on future (disallowed) slots, 0 on allowed: added to scores
    q_idx = np.arange(256)[None, :]
    masks = np.stack(
        [np.where(q_idx >= j * 128 + k_idx, 0.0, -30.0).astype(np.float32)
         for j in range(2)],
        axis=1).astype(bf)                             # [128, 2, 256]

    hidT = [np.ascontiguousarray(
        hidden_states[b].reshape(S, HIDDEN).T).astype(bf) for b in range(B)]

    in_maps = []
    for c in range(NCORES):
        b, g = divmod(c, 4)
        # q rows: heads 8g..8g+7, pi-permuted within each head
        wq_rows = np.concatenate(
            [Wq[(8 * g + h) * 128:(8 * g + h) * 128 + 128][PERM]
             for h in range(QT)], axis=0)              # [1024, 4096]
        # folded nope-key rows (pi nope order == Wupk row order per head)
        wkc = Wupk[128 * g:128 * g + 128] @ Wdk        # [128, 4096]
        # rope-key rows (pi rope order == Wkr row order per head)
        wkr = Wkr[128 * g:128 * g + 128]               # [128, 4096]
        # folded v rows, canonical head-dim order
        wv = Wupv[256 * g:256 * g + 256] @ Wdk         # [256, 4096]
        w1 = np.ascontiguousarray(
            np.concatenate([wq_rows, wkc, wkr, wv], axis=0).T).astype(bf)
        wo_c = np.ascontiguousarray(
            Wo[:, QR * g:QR * (g + 1)].T).astype(bf)   # [1024, 4096]
        in_maps.append({
            "hidT": hidT[b], "w1": w1, "wo_t": wo_c,
            "qcos": qcos, "qsin": qsin, "kcos": kcos, "ksin": ksin,
            "masks": masks, "ones": np.ones((128, 1), np.float32),
            "ident": np.eye(128, dtype=np.float32).astype(bf),
        })
    return in_maps


def combine_outputs(results):
    outs = []
    for b in range(B):
        o = results[4 * b]["out_part"].astype(np.float32)
        for g in range(1, 4):
            o = o + results[4 * b + g]["out_part"]
        outs.append(o)
    return np.stack(outs, axis=0).reshape(B, S, HIDDEN).astype(np.float32)


_NC_CACHE = {}


def _get_program(key=0):
    if key not in _NC_CACHE:
        _NC_CACHE[key] = build_program()
    return _NC_CACHE[key]


def kernel(hidden_states, Wq, Wkr, Wdk, Wupk, Wupv, Wo):
    from concourse.bass_utils import run_bass_kernel_spmd

    in_maps = make_in_maps(hidden_states, Wq, Wkr, Wdk, Wupk, Wupv, Wo)
    nc = _get_program()
    res = run_bass_kernel_spmd(nc, in_maps, list(range(NCORES)))
    return combine_outputs(res.results)
